# revision 45
# baseline (speedup 1.0000x reference)
"""Trainium2 Bass kernel for nn_BlurLayer (B=128, 224x224x3, per-sample
rotated-line motion blur, SAME depthwise conv).

Self-contained: kernel(**inputs) -> np.ndarray. Shards the batch over 8
NeuronCores (pure data parallel: 16 samples per core), compiles + runs one
SPMD Bass program via concourse.bass_utils.run_bass_kernel_spmd, gathers
the full output.

Method: the rotated blur kernel's nonzero taps all equal 1/size and form a
digitized line. The conv is computed as G matmuls per 112-row output block:
out[r, u] += W_g[p, r] * window[p, u + 3g], where the window rows carry a
per-row horizontal shift sigma(m) and the psum columns carry a per-row
output shift omega(Y) (both baked host-side into the blob / unshard, so
all device access patterns are static). sigma and omega are chosen per
sample by a Bellman-Ford difference-constraint solver to minimize G (the
number of distinct shifted column groups needed to cover every tap) -- for
any line angle this lands at G in {1,2,3} instead of the raw column span.
The masks W_g are arbitrary per-sample 0/1 matrices (clipped at the block
edges, which implements the vertical SAME padding). The image is split
into two fp8e4m3 planes (hi = fp8(x), lo = fp8(x - hi)) so each matmul
runs in DoubleRow perf mode; 1/size scaling happens in the PSUM->SBUF
copies, split between the Scalar and Vector engines. Each slot's windows +
mask table arrive in one DMA; a post-compile pass drops back-to-back
reloads of identical PE weights; outputs DMA out per half-block to shrink
the tail.
"""

import math

import numpy as np
import ml_dtypes

MAXK = 32
H = W = 224
C = 3
WC = W * C                  # 672
PAD_LO = (MAXK - 1) // 2    # 15
PIMG_PAD = 800              # left zero margin (elems) of padded image row
PIMG_W = PIMG_PAD + WC + 800

FP8 = ml_dtypes.float8_e4m3


def dedupe_ldweights(nc):
    """Replace an InstLdweights whose weights AP is identical to the
    immediately-preceding one (per block) with an InstNoOp carrying its
    sync_info: the PE array still holds those weights, so the reload is
    pure Tensor-queue overhead (~158ns each)."""
    import concourse.mybir as mybir
    n = 0
    for fn in nc.m.functions:
        for blk in fn.blocks:
            prev_key = None
            new_insts = []
            for inst in blk.instructions:
                if isinstance(inst, mybir.InstLdweights):
                    key = (repr(inst.ins), repr(getattr(inst, "perf_mode", None)),
                           repr(getattr(inst, "is_transpose", None)))
                    if key == prev_key:
                        n += 1
                        inst = mybir.InstNoOp(
                            name=f"{inst.name}-ldwdedup",
                            engine=inst.engine,
                            ins=[], outs=[],
                            sync_info=inst.sync_info,
                        )
                    else:
                        prev_key = key
                new_insts.append(inst)
            blk.instructions = new_insts
    return n


# ---------------------------------------------------------------- host math
def rotate_nearest_np(img, rad):
    K = img.shape[0]
    cos, sin = np.cos(rad), np.sin(rad)
    coords = np.arange(K, dtype=np.float32)
    yy, xx = np.meshgrid(coords, coords, indexing="ij")
    e = np.float32(K - 1)
    x_off = (e - (cos * e - sin * e)) * 0.5
    y_off = (e - (sin * e + cos * e)) * 0.5
    sx = cos * xx - sin * yy + x_off
    sy = sin * xx + cos * yy + y_off
    ix = np.round(sx).astype(np.int32)
    iy = np.round(sy).astype(np.int32)
    valid = (ix >= 0) & (ix < K) & (iy >= 0) & (iy < K)
    g = img[np.clip(iy, 0, K - 1), np.clip(ix, 0, K - 1)]
    return np.where(valid, g, np.float32(0.0))


def tap_pairs(ker):
    """Valid (m, Y, kx) triples: img row m = Y + ky - 15 for out row Y and
    tap (ky, kx), both m and Y in [0, 224)."""
    ys, xs = np.nonzero(ker)
    Y = np.arange(H)
    M, YY, KX = [], [], []
    for ky, kx in zip(ys, xs):
        m = Y + ky - 15
        ok = (m >= 0) & (m < H)
        M.append(m[ok])
        YY.append(Y[ok])
        KX.append(np.full(int(ok.sum()), kx))
    return np.concatenate(M), np.concatenate(YY), np.concatenate(KX)


def solve_shear(ker, max_rounds=400):
    """Choose integer shift profiles sigma (img rows) / nu (out rows)
    minimizing G = #values of g = kx - sigma[m] + nu[Y] over all taps.
    Difference-constraint feasibility via vectorized Bellman-Ford.
    Returns (G, sigma[224], nu[224], c) with g - c in [0, G)."""
    M, Y, KX = tap_pairs(ker)
    key = M * H + Y
    order = np.argsort(key)
    key_s, kx_s = key[order], KX[order]
    uk, idx = np.unique(key_s, return_index=True)
    lo = np.minimum.reduceat(kx_s, idx)
    hi = np.maximum.reduceat(kx_s, idx)
    um, uy = uk // H, uk % H
    G0 = int((hi - lo).max()) + 1
    for G in range(G0, 40):
        src = np.concatenate([224 + uy, um])
        dst = np.concatenate([um, 224 + uy])
        w = np.concatenate([lo, (G - 1) - hi]).astype(np.int64)
        dist = np.zeros(448, np.int64)
        ok = False
        for _ in range(max_rounds):
            nd = dist.copy()
            np.minimum.at(nd, dst, dist[src] + w)
            if np.array_equal(nd, dist):
                ok = True
                break
            dist = nd
        if ok:
            sigma, nu = dist[:224], dist[224:]
            g = KX - sigma[M] + nu[Y]
            c = int(g.min())
            assert int(g.max()) - c + 1 <= G
            return int(g.max()) - c + 1, sigma, nu, c
    raise RuntimeError("no feasible G")


def sample_plan(tbl_ch0, amt_b, ang_b):
    """-> dict with scale, taps, G, sigma, nu, c, wout, transposed."""
    rad = np.float32(ang_b * math.pi / 180.0)
    ker0 = rotate_nearest_np(tbl_ch0[amt_b], rad)
    ys, xs = np.nonzero(ker0)
    scale = float(ker0[ys[0], xs[0]])
    best = None
    for tr in (False, True):
        km = ker0.T if tr else ker0
        G, sigma, nu, c = solve_shear(km)
        w0 = int(nu[0:112].max() - nu[0:112].min())
        w1 = int(nu[112:224].max() - nu[112:224].min())
        wout = WC + 3 * max(w0, w1)
        cost = 2 * G * wout
        if best is None or cost < best["cost"]:
            best = dict(cost=cost, G=G, sigma=sigma, nu=nu, c=c, wout=wout,
                        transposed=tr, ker=km, scale=np.float32(scale))
    return best


def prepare_host(x, kernels_table, amt, angles, n_cores=8):
    B = x.shape[0]
    assert B % n_cores == 0
    slots = B // n_cores
    tbl_ch0 = np.ascontiguousarray(kernels_table[:, :, :, 0])

    plans = [sample_plan(tbl_ch0, int(amt[b]), int(angles[b]))
             for b in range(B)]
    Gs = np.array([p["G"] for p in plans])
    wos = np.array([p["wout"] for p in plans])

    # slot packing: G uniform-ish per slot (sort by G then wout, rows of 8)
    order = np.lexsort((-wos, -Gs))
    asg = order.reshape(slots, n_cores)

    slotG = np.array([Gs[asg[j]].max() for j in range(slots)])
    slotW = np.array([wos[asg[j]].max() for j in range(slots)])

    # schedule: a light slot first (its input DMA completes fastest, so
    # the PE starts early), the lightest last (small tail), heavy middle.
    slot_cost = 2 * slotG * slotW
    o = np.argsort(-slot_cost, kind="stable")   # heavy .. light
    sched = np.concatenate([[o[-2]], o[:-2], [o[-1]]])
    asg = asg[sched]
    slotG = slotG[sched]
    slotW = slotW[sched]

    gmax = slotG
    wout = slotW
    wprime = ((wout + 3 * gmax + 7) // 8) * 8
    blobw = 4 * wprime + 224 * gmax            # 4 image planes + mask table
    col_base = np.concatenate([[0], np.cumsum(blobw)])[:-1]
    totbw = int(blobw.sum())
    out_base = np.concatenate([[0], np.cumsum(2 * 112 * wout)])[:-1]
    totout = int((2 * 112 * wout).sum())

    # fp8 hi/lo planes of the full batch
    x8hi = x.astype(FP8)
    xlo = x - x8hi.astype(np.float32)
    x8lo = xlo.astype(FP8)

    in_maps = []
    mapping = np.zeros((n_cores, slots), np.int64)
    omega_all = np.zeros((n_cores, slots, 2, 112), np.int64)
    for cidx in range(n_cores):
        blob = np.zeros((128, totbw), FP8)
        scl = np.zeros((128, slots), np.float32)
        for j in range(slots):
            b = int(asg[j, cidx])
            p = plans[b]
            G = int(gmax[j])
            Wp = int(wprime[j])
            base = int(col_base[j])
            mapping[cidx, j] = b
            scl[:, j] = p["scale"]
            sigma, nu, c = p["sigma"], p["nu"], p["c"]

            if p["transposed"]:
                hi = np.ascontiguousarray(x8hi[b].transpose(1, 0, 2)).reshape(H, WC)
                lo = np.ascontiguousarray(x8lo[b].transpose(1, 0, 2)).reshape(H, WC)
            else:
                hi = x8hi[b].reshape(H, WC)
                lo = x8lo[b].reshape(H, WC)
            phi = np.zeros((H, PIMG_W), FP8)
            plo = np.zeros((H, PIMG_W), FP8)
            phi[:, PIMG_PAD:PIMG_PAD + WC] = hi
            plo[:, PIMG_PAD:PIMG_PAD + WC] = lo

            for hb, (R, S) in enumerate(((0, 0), (112, 96))):
                numax = int(nu[R:R + 112].max())
                omega_all[cidx, j, hb] = numax - nu[R:R + 112]
                V0 = PIMG_PAD + 3 * (c - PAD_LO - numax)
                rows = np.arange(128)
                cols = V0 + 3 * sigma[S + rows]
                assert cols.min() >= 0 and cols.max() + Wp <= PIMG_W, \
                    (b, hb, cols.min(), cols.max(), Wp)
                for pl, pimg in enumerate((phi, plo)):
                    dst = base + (2 * hb + pl) * Wp
                    win = np.zeros((128, Wp), FP8)
                    for pp in range(128):
                        win[pp] = pimg[S + pp, cols[pp]:cols[pp] + Wp]
                    blob[:, dst:dst + Wp] = win

            # mask table: [128, G, 2(hb), 112] fp8
            wtb = base + 4 * Wp
            wcols = np.zeros((128, G, 2, 112), np.float32)
            ys, xs2 = np.nonzero(p["ker"])
            r = np.arange(112)
            for ky, kx in zip(ys, xs2):
                for hb, (R, S) in enumerate(((0, 0), (112, 96))):
                    m = R + r + ky - PAD_LO
                    pr = m - S
                    ok = (pr >= 0) & (pr < 128) & (m >= 0) & (m < H)
                    if not ok.any():
                        continue
                    g = (kx - sigma[m[ok]] + nu[R + r[ok]]) - c
                    assert g.min() >= 0 and g.max() < G, (b, hb, g.min(), g.max(), G)
                    wcols[pr[ok], g, hb, r[ok]] = 1.0
            blob[:, wtb:wtb + 224 * G] = np.ascontiguousarray(wcols).reshape(
                128, 224 * G).astype(FP8)
        in_maps.append({"ximg": blob, "scl": scl})

    meta = {
        "slots": slots,
        "gmax": [int(v) for v in gmax],
        "wout": [int(v) for v in wout],
        "wprime": [int(v) for v in wprime],
        "blobw": [int(v) for v in blobw],
        "col_base": [int(v) for v in col_base],
        "out_base": [int(v) for v in out_base],
        "totbw": totbw,
        "totout": totout,
        "mapping": mapping,
        "omega": omega_all,
        "transposed": np.array([p["transposed"] for p in plans]),
    }
    return meta, in_maps


def _chunks(wout):
    """Split a result width into <=512-col PSUM chunks."""
    n = -(-wout // 512)
    w = -(-wout // n)
    out = []
    off = 0
    while off < wout:
        cc = min(w, wout - off)
        out.append((off, cc))
        off += cc
    return out


# ---------------------------------------------------------------- device IR
def build_program(meta):
    import concourse.bacc as bacc
    import concourse.mybir as mybir
    from concourse.tile import TileContext
    from bass_rust import VecI64Pair

    fp8 = mybir.dt.float8e4
    slots = meta["slots"]

    nc = bacc.Bacc("TRN2")
    ximg = nc.dram_tensor("ximg", [128, meta["totbw"]], fp8, kind="ExternalInput")
    scl = nc.dram_tensor("scl", [128, slots], mybir.dt.float32,
                         kind="ExternalInput")
    out = nc.dram_tensor("out", [1, meta["totout"]], mybir.dt.float16,
                         kind="ExternalOutput")

    def strided(tile, dims, offset):
        ap = tile[:, 0:1].copy()
        ap.ap = VecI64Pair(dims)
        ap.offset = offset
        return ap

    with TileContext(nc) as tc:
        with tc.tile_pool(name="const", bufs=1) as cpool, \
             tc.tile_pool(name="img", bufs=10) as ipool, \
             tc.tile_pool(name="res", bufs=10) as rpool, \
             tc.tile_pool(name="warm", bufs=1) as wmpool, \
             tc.tile_pool(name="ps0", bufs=2, space="PSUM") as pw0, \
             tc.tile_pool(name="ps1", bufs=2, space="PSUM") as pw1, \
             tc.tile_pool(name="ps2", bufs=2, space="PSUM") as pw2, \
             tc.tile_pool(name="psw", bufs=1, space="PSUM") as pww:

            st = cpool.tile([128, slots], mybir.dt.float32)
            nc.scalar.dma_start(out=st, in_=scl[:, :])

            # PE warmup: junk matmuls keep the PE busy through the HAM
            # activity window while the first blobs stream in, so the real
            # matmuls start at 2.4 GHz instead of 1.2.
            wm = wmpool.tile([128, 624], fp8)
            nc.vector.memset(wm, 0.0)
            wps = pww.tile([112, 512], mybir.dt.float32)
            for _ in range(7):
                nc.tensor.matmul(wps, lhsT=wm[:, 0:112], rhs=wm[:, 112:624],
                                 start=True, stop=True)

            wpools = [pw0, pw1, pw2]
            for j in range(slots):
                G = meta["gmax"][j]
                WO = meta["wout"][j]
                Wp = meta["wprime"][j]
                BW = meta["blobw"][j]
                base = meta["col_base"][j]
                obase = meta["out_base"][j]
                ch = _chunks(WO)
                blob = ipool.tile([128, BW], fp8, tag="blob", name="blob")
                wtb = 4 * Wp
                # windows and masks as separate DMAs, both scheduled on
                # sync (keeps the tile scheduler's matmul ordering);
                # move_mask_dmas() flips the mask copies to the idle
                # scalar ring after compile
                nc.sync.dma_start(out=blob[:, 0:wtb],
                                  in_=ximg[:, base:base + wtb])
                nc.sync.dma_start(out=blob[:, wtb:BW],
                                  in_=ximg[:, base + wtb:base + BW])

                sc = st[0:112, j:j + 1]
                for hb in (0, 1):
                    rt = rpool.tile([112, WO], mybir.dt.float16, tag=f"rt{hb}",
                                    name=f"rt{hb}")
                    psums = [wpools[wh].tile([112, ch[wh][1]], mybir.dt.float32,
                                             tag=f"ps{wh}", name=f"ps{wh}")
                             for wh in range(len(ch))]
                    for g in range(G):
                        # same mask for both fp8 planes (hi, lo)
                        lhs = strided(blob, [[BW, 128], [0, 2], [1, 112]],
                                      wtb + 224 * g + 112 * hb)
                        for wh in range(len(ch)):
                            # planes (hi, lo) of window hb at column shift 3g
                            rhs = strided(
                                blob, [[BW, 128], [Wp, 2], [1, ch[wh][1]]],
                                2 * hb * Wp + 3 * g + ch[wh][0])
                            nc.tensor.matmul(
                                psums[wh], lhsT=lhs, rhs=rhs,
                                start=(g == 0), stop=(g == G - 1),
                                perf_mode=mybir.MatmulPerfMode.DoubleRow)
                    for wh in range(len(ch)):
                        dstc = rt[:, ch[wh][0]:ch[wh][0] + ch[wh][1]]
                        if wh == 0:
                            nc.scalar.activation(
                                out=dstc, in_=psums[wh],
                                func=mybir.ActivationFunctionType.Copy,
                                scale=sc)
                        else:
                            nc.vector.tensor_scalar_mul(out=dstc,
                                                        in0=psums[wh],
                                                        scalar1=sc)
                    src = strided(rt, [[WO, 112], [1, WO]], 0)
                    dst = out[0, 0:1].copy()
                    dst.ap = VecI64Pair([[WO, 112], [1, WO]])
                    dst.offset = obase + hb * 112 * WO
                    nc.gpsimd.dma_start(out=dst, in_=src)
    return nc


def move_mask_dmas(nc):
    """Post-compile: flip the (small) mask-table input DMAs from the sync
    queue to the scalar queue so they ride the otherwise-idle scalar DMA
    ring. They carry no waits beyond pool reuse, so the queue swap cannot
    deadlock; semaphores are global and move with the instruction."""
    import concourse.mybir as mybir
    moved = 0
    for fn in nc.m.functions:
        for blk in fn.blocks:
            for inst in blk.instructions:
                if not isinstance(inst, mybir.InstDMACopy):
                    continue
                if inst.engine != mybir.EngineType.SP:
                    continue
                o = inst.outs[0]
                if not str(getattr(o, "memref", "")).startswith("blob"):
                    continue
                dims = list(o.ap)
                width = dims[-1][1] if dims else 0
                if width <= 224 * 4:          # mask table (<= 224*G)
                    inst.engine = mybir.EngineType.Activation
                    moved += 1
    return moved


def run_cores(meta, in_maps, trace=False):
    from concourse.bass_utils import run_bass_kernel_spmd

    nc = build_program(meta)
    nc.compile()
    dedupe_ldweights(nc)
    move_mask_dmas(nc)
    res = run_bass_kernel_spmd(nc, in_maps, core_ids=list(range(len(in_maps))),
                               trace=trace)
    return res


def unshard(meta, results):
    B = meta["mapping"].size
    out = np.zeros((B, H, W, C), np.float32)
    for cidx, r in enumerate(results):
        o = np.asarray(r["out"], np.float32).reshape(-1)
        for j in range(meta["slots"]):
            b = meta["mapping"][cidx, j]
            WO = meta["wout"][j]
            t = o[meta["out_base"][j]:meta["out_base"][j] + 2 * 112 * WO]
            t = t.reshape(2, 112, WO)
            img = np.zeros((H, WC), np.float32)
            om = meta["omega"][cidx, j]
            for hb in (0, 1):
                for r_ in range(112):
                    u = 3 * int(om[hb, r_])
                    img[112 * hb + r_] = t[hb, r_, u:u + WC]
            img = img.reshape(H, W, C)
            if meta["transposed"][b]:
                img = img.transpose(1, 0, 2)
            out[b] = img
    return out


def kernel(x, kernels_table, amt, angles):
    x = np.asarray(x, np.float32)
    kernels_table = np.asarray(kernels_table, np.float32)
    amt = np.asarray(amt)
    angles = np.asarray(angles)
    meta, in_maps = prepare_host(x, kernels_table, amt, angles)
    res = run_cores(meta, in_maps)
    return unshard(meta, res.results)


# revision 46
# speedup vs baseline: 1.2869x; 1.2869x over previous
"""Trainium2 Bass kernel for nn_BlurLayer (B=128, 224x224x3, per-sample
rotated-line motion blur, SAME depthwise conv).

Self-contained: kernel(**inputs) -> np.ndarray. Shards the batch over 8
NeuronCores (pure data parallel: 16 samples per core), compiles + runs one
SPMD Bass program via concourse.bass_utils.run_bass_kernel_spmd, gathers
the full output.

Method: the rotated blur kernel's nonzero taps all equal 1/size and form a
digitized line. The conv is computed as G matmuls per 112-row output block:
out[r, u] += W_g[p, r] * window[p, u + 3g], where the window rows carry a
per-row horizontal shift sigma(m) and the psum columns carry a per-row
output shift omega(Y) (both baked host-side into the blob / unshard, so
all device access patterns are static). sigma and omega are chosen per
sample by a Bellman-Ford difference-constraint solver to minimize G (the
number of distinct shifted column groups needed to cover every tap) -- for
any line angle this lands at G in {1,2,3} instead of the raw column span.
The masks W_g are arbitrary per-sample 0/1 matrices (clipped at the block
edges, which implements the vertical SAME padding). The image is split
into two fp8e4m3 planes (hi = fp8(x), lo = fp8(x - hi)) so each matmul
runs in DoubleRow perf mode; 1/size scaling happens in the PSUM->SBUF
copies, split between the Scalar and Vector engines. Each slot's windows +
mask table arrive in one DMA; a post-compile pass drops back-to-back
reloads of identical PE weights; outputs DMA out per half-block to shrink
the tail.
"""

import math

import numpy as np
import ml_dtypes

MAXK = 32
H = W = 224
C = 3
WC = W * C                  # 672
PAD_LO = (MAXK - 1) // 2    # 15
PIMG_PAD = 800              # left zero margin (elems) of padded image row
PIMG_W = PIMG_PAD + WC + 800

FP8 = ml_dtypes.float8_e4m3


def dedupe_ldweights(nc):
    """Replace an InstLdweights whose weights AP is identical to the
    immediately-preceding one (per block) with an InstNoOp carrying its
    sync_info: the PE array still holds those weights, so the reload is
    pure Tensor-queue overhead (~158ns each)."""
    import concourse.mybir as mybir
    n = 0
    for fn in nc.m.functions:
        for blk in fn.blocks:
            prev_key = None
            new_insts = []
            for inst in blk.instructions:
                if isinstance(inst, mybir.InstLdweights):
                    key = (repr(inst.ins), repr(getattr(inst, "perf_mode", None)),
                           repr(getattr(inst, "is_transpose", None)))
                    if key == prev_key:
                        n += 1
                        inst = mybir.InstNoOp(
                            name=f"{inst.name}-ldwdedup",
                            engine=inst.engine,
                            ins=[], outs=[],
                            sync_info=inst.sync_info,
                        )
                    else:
                        prev_key = key
                new_insts.append(inst)
            blk.instructions = new_insts
    return n


# ---------------------------------------------------------------- host math
def rotate_nearest_np(img, rad):
    K = img.shape[0]
    cos, sin = np.cos(rad), np.sin(rad)
    coords = np.arange(K, dtype=np.float32)
    yy, xx = np.meshgrid(coords, coords, indexing="ij")
    e = np.float32(K - 1)
    x_off = (e - (cos * e - sin * e)) * 0.5
    y_off = (e - (sin * e + cos * e)) * 0.5
    sx = cos * xx - sin * yy + x_off
    sy = sin * xx + cos * yy + y_off
    ix = np.round(sx).astype(np.int32)
    iy = np.round(sy).astype(np.int32)
    valid = (ix >= 0) & (ix < K) & (iy >= 0) & (iy < K)
    g = img[np.clip(iy, 0, K - 1), np.clip(ix, 0, K - 1)]
    return np.where(valid, g, np.float32(0.0))


def tap_pairs(ker):
    """Valid (m, Y, kx) triples: img row m = Y + ky - 15 for out row Y and
    tap (ky, kx), both m and Y in [0, 224)."""
    ys, xs = np.nonzero(ker)
    Y = np.arange(H)
    M, YY, KX = [], [], []
    for ky, kx in zip(ys, xs):
        m = Y + ky - 15
        ok = (m >= 0) & (m < H)
        M.append(m[ok])
        YY.append(Y[ok])
        KX.append(np.full(int(ok.sum()), kx))
    return np.concatenate(M), np.concatenate(YY), np.concatenate(KX)


def solve_shear(ker, max_rounds=400):
    """Choose integer shift profiles sigma (img rows) / nu (out rows)
    minimizing G = #values of g = kx - sigma[m] + nu[Y] over all taps.
    Difference-constraint feasibility via vectorized Bellman-Ford.
    Returns (G, sigma[224], nu[224], c) with g - c in [0, G)."""
    M, Y, KX = tap_pairs(ker)
    key = M * H + Y
    order = np.argsort(key)
    key_s, kx_s = key[order], KX[order]
    uk, idx = np.unique(key_s, return_index=True)
    lo = np.minimum.reduceat(kx_s, idx)
    hi = np.maximum.reduceat(kx_s, idx)
    um, uy = uk // H, uk % H
    G0 = int((hi - lo).max()) + 1
    for G in range(G0, 40):
        src = np.concatenate([224 + uy, um])
        dst = np.concatenate([um, 224 + uy])
        w = np.concatenate([lo, (G - 1) - hi]).astype(np.int64)
        dist = np.zeros(448, np.int64)
        ok = False
        for _ in range(max_rounds):
            nd = dist.copy()
            np.minimum.at(nd, dst, dist[src] + w)
            if np.array_equal(nd, dist):
                ok = True
                break
            dist = nd
        if ok:
            sigma, nu = dist[:224], dist[224:]
            g = KX - sigma[M] + nu[Y]
            c = int(g.min())
            assert int(g.max()) - c + 1 <= G
            return int(g.max()) - c + 1, sigma, nu, c
    raise RuntimeError("no feasible G")


def sample_plan(tbl_ch0, amt_b, ang_b):
    """-> dict with scale, taps, G, sigma, nu, c, wout, transposed."""
    rad = np.float32(ang_b * math.pi / 180.0)
    ker0 = rotate_nearest_np(tbl_ch0[amt_b], rad)
    ys, xs = np.nonzero(ker0)
    scale = float(ker0[ys[0], xs[0]])
    best = None
    for tr in (False, True):
        km = ker0.T if tr else ker0
        G, sigma, nu, c = solve_shear(km)
        w0 = int(nu[0:112].max() - nu[0:112].min())
        w1 = int(nu[112:224].max() - nu[112:224].min())
        wout = WC + 3 * max(w0, w1)
        cost = 2 * G * wout
        if best is None or cost < best["cost"]:
            best = dict(cost=cost, G=G, sigma=sigma, nu=nu, c=c, wout=wout,
                        transposed=tr, ker=km, scale=np.float32(scale))
    return best


def prepare_host(x, kernels_table, amt, angles, n_cores=8):
    B = x.shape[0]
    assert B % n_cores == 0
    slots = B // n_cores
    tbl_ch0 = np.ascontiguousarray(kernels_table[:, :, :, 0])

    plans = [sample_plan(tbl_ch0, int(amt[b]), int(angles[b]))
             for b in range(B)]
    Gs = np.array([p["G"] for p in plans])
    wos = np.array([p["wout"] for p in plans])

    # slot packing: G uniform-ish per slot (sort by G then wout, rows of 8)
    order = np.lexsort((-wos, -Gs))
    asg = order.reshape(slots, n_cores)

    slotG = np.array([Gs[asg[j]].max() for j in range(slots)])
    slotW = np.array([wos[asg[j]].max() for j in range(slots)])

    # schedule: a light slot first (its input DMA completes fastest, so
    # the PE starts early), the lightest last (small tail), heavy middle.
    slot_cost = 2 * slotG * slotW
    o = np.argsort(-slot_cost, kind="stable")   # heavy .. light
    sched = np.concatenate([[o[-2]], o[:-2], [o[-1]]])
    asg = asg[sched]
    slotG = slotG[sched]
    slotW = slotW[sched]

    gmax = slotG
    wout = slotW
    wprime = ((wout + 3 * gmax + 7) // 8) * 8
    blobw = 4 * wprime + 224 * gmax            # 4 image planes + mask table
    col_base = np.concatenate([[0], np.cumsum(blobw)])[:-1]
    totbw = int(blobw.sum())
    out_base = np.concatenate([[0], np.cumsum(2 * 112 * wout)])[:-1]
    totout = int((2 * 112 * wout).sum())

    # fp8 hi/lo planes of the full batch
    x8hi = x.astype(FP8)
    xlo = x - x8hi.astype(np.float32)
    x8lo = xlo.astype(FP8)

    in_maps = []
    mapping = np.zeros((n_cores, slots), np.int64)
    omega_all = np.zeros((n_cores, slots, 2, 112), np.int64)
    for cidx in range(n_cores):
        blob = np.zeros((128, totbw), FP8)
        scl = np.zeros((128, slots), np.float32)
        for j in range(slots):
            b = int(asg[j, cidx])
            p = plans[b]
            G = int(gmax[j])
            Wp = int(wprime[j])
            base = int(col_base[j])
            mapping[cidx, j] = b
            scl[:, j] = p["scale"]
            sigma, nu, c = p["sigma"], p["nu"], p["c"]

            if p["transposed"]:
                hi = np.ascontiguousarray(x8hi[b].transpose(1, 0, 2)).reshape(H, WC)
                lo = np.ascontiguousarray(x8lo[b].transpose(1, 0, 2)).reshape(H, WC)
            else:
                hi = x8hi[b].reshape(H, WC)
                lo = x8lo[b].reshape(H, WC)
            phi = np.zeros((H, PIMG_W), FP8)
            plo = np.zeros((H, PIMG_W), FP8)
            phi[:, PIMG_PAD:PIMG_PAD + WC] = hi
            plo[:, PIMG_PAD:PIMG_PAD + WC] = lo

            for hb, (R, S) in enumerate(((0, 0), (112, 96))):
                numax = int(nu[R:R + 112].max())
                omega_all[cidx, j, hb] = numax - nu[R:R + 112]
                V0 = PIMG_PAD + 3 * (c - PAD_LO - numax)
                rows = np.arange(128)
                cols = V0 + 3 * sigma[S + rows]
                assert cols.min() >= 0 and cols.max() + Wp <= PIMG_W, \
                    (b, hb, cols.min(), cols.max(), Wp)
                for pl, pimg in enumerate((phi, plo)):
                    dst = base + (2 * hb + pl) * Wp
                    win = np.zeros((128, Wp), FP8)
                    for pp in range(128):
                        win[pp] = pimg[S + pp, cols[pp]:cols[pp] + Wp]
                    blob[:, dst:dst + Wp] = win

            # mask table: [128, G, 2(hb), 112] fp8
            wtb = base + 4 * Wp
            wcols = np.zeros((128, G, 2, 112), np.float32)
            ys, xs2 = np.nonzero(p["ker"])
            r = np.arange(112)
            for ky, kx in zip(ys, xs2):
                for hb, (R, S) in enumerate(((0, 0), (112, 96))):
                    m = R + r + ky - PAD_LO
                    pr = m - S
                    ok = (pr >= 0) & (pr < 128) & (m >= 0) & (m < H)
                    if not ok.any():
                        continue
                    g = (kx - sigma[m[ok]] + nu[R + r[ok]]) - c
                    assert g.min() >= 0 and g.max() < G, (b, hb, g.min(), g.max(), G)
                    wcols[pr[ok], g, hb, r[ok]] = 1.0
            blob[:, wtb:wtb + 224 * G] = np.ascontiguousarray(wcols).reshape(
                128, 224 * G).astype(FP8)
        in_maps.append({"ximg": blob, "scl": scl})

    meta = {
        "slots": slots,
        "gmax": [int(v) for v in gmax],
        "wout": [int(v) for v in wout],
        "wprime": [int(v) for v in wprime],
        "blobw": [int(v) for v in blobw],
        "col_base": [int(v) for v in col_base],
        "out_base": [int(v) for v in out_base],
        "totbw": totbw,
        "totout": totout,
        "mapping": mapping,
        "omega": omega_all,
        "transposed": np.array([p["transposed"] for p in plans]),
    }
    return meta, in_maps


def _chunks(wout):
    """Split a result width into <=512-col PSUM chunks."""
    n = -(-wout // 512)
    w = -(-wout // n)
    out = []
    off = 0
    while off < wout:
        cc = min(w, wout - off)
        out.append((off, cc))
        off += cc
    return out


# ---------------------------------------------------------------- device IR
def build_program(meta):
    import concourse.bacc as bacc
    import concourse.mybir as mybir
    from concourse.tile import TileContext
    from bass_rust import VecI64Pair

    fp8 = mybir.dt.float8e4
    slots = meta["slots"]

    nc = bacc.Bacc("TRN2")
    ximg = nc.dram_tensor("ximg", [128, meta["totbw"]], fp8, kind="ExternalInput")
    scl = nc.dram_tensor("scl", [128, slots], mybir.dt.float32,
                         kind="ExternalInput")
    out = nc.dram_tensor("out", [1, meta["totout"]], mybir.dt.float16,
                         kind="ExternalOutput")

    def strided(tile, dims, offset):
        ap = tile[:, 0:1].copy()
        ap.ap = VecI64Pair(dims)
        ap.offset = offset
        return ap

    with TileContext(nc) as tc:
        with tc.tile_pool(name="const", bufs=1) as cpool, \
             tc.tile_pool(name="img", bufs=10) as ipool, \
             tc.tile_pool(name="res", bufs=10) as rpool, \
             tc.tile_pool(name="warm", bufs=1) as wmpool, \
             tc.tile_pool(name="ps0", bufs=2, space="PSUM") as pw0, \
             tc.tile_pool(name="ps1", bufs=2, space="PSUM") as pw1, \
             tc.tile_pool(name="ps2", bufs=2, space="PSUM") as pw2, \
             tc.tile_pool(name="psw", bufs=1, space="PSUM") as pww:

            st = cpool.tile([128, slots], mybir.dt.float32)
            nc.scalar.dma_start(out=st, in_=scl[:, :])

            # PE warmup: junk matmuls keep the PE busy through the HAM
            # activity window while the first blobs stream in, so the real
            # matmuls start at 2.4 GHz instead of 1.2.
            wm = wmpool.tile([128, 624], fp8)
            nc.vector.memset(wm, 0.0)
            wps = pww.tile([112, 512], mybir.dt.float32)
            for _ in range(7):
                nc.tensor.matmul(wps, lhsT=wm[:, 0:112], rhs=wm[:, 112:624],
                                 start=True, stop=True)

            wpools = [pw0, pw1, pw2]
            for j in range(slots):
                G = meta["gmax"][j]
                WO = meta["wout"][j]
                Wp = meta["wprime"][j]
                BW = meta["blobw"][j]
                base = meta["col_base"][j]
                obase = meta["out_base"][j]
                ch = _chunks(WO)
                blob = ipool.tile([128, BW], fp8, tag="blob", name="blob")
                wtb = 4 * Wp
                nc.sync.dma_start(out=blob, in_=ximg[:, base:base + BW])

                sc = st[0:112, j:j + 1]
                for hb in (0, 1):
                    rt = rpool.tile([112, WO], mybir.dt.float16, tag=f"rt{hb}",
                                    name=f"rt{hb}")
                    psums = [wpools[wh].tile([112, ch[wh][1]], mybir.dt.float32,
                                             tag=f"ps{wh}", name=f"ps{wh}")
                             for wh in range(len(ch))]
                    for g in range(G):
                        # same mask for both fp8 planes (hi, lo)
                        lhs = strided(blob, [[BW, 128], [0, 2], [1, 112]],
                                      wtb + 224 * g + 112 * hb)
                        for wh in range(len(ch)):
                            # planes (hi, lo) of window hb at column shift 3g
                            rhs = strided(
                                blob, [[BW, 128], [Wp, 2], [1, ch[wh][1]]],
                                2 * hb * Wp + 3 * g + ch[wh][0])
                            nc.tensor.matmul(
                                psums[wh], lhsT=lhs, rhs=rhs,
                                start=(g == 0), stop=(g == G - 1),
                                perf_mode=mybir.MatmulPerfMode.DoubleRow)
                    for wh in range(len(ch)):
                        dstc = rt[:, ch[wh][0]:ch[wh][0] + ch[wh][1]]
                        if wh == 0:
                            nc.scalar.activation(
                                out=dstc, in_=psums[wh],
                                func=mybir.ActivationFunctionType.Copy,
                                scale=sc)
                        else:
                            nc.vector.tensor_scalar_mul(out=dstc,
                                                        in0=psums[wh],
                                                        scalar1=sc)
                    src = strided(rt, [[WO, 112], [1, WO]], 0)
                    dst = out[0, 0:1].copy()
                    dst.ap = VecI64Pair([[WO, 112], [1, WO]])
                    dst.offset = obase + hb * 112 * WO
                    nc.gpsimd.dma_start(out=dst, in_=src)
    return nc


def run_cores(meta, in_maps, trace=False):
    from concourse.bass_utils import run_bass_kernel_spmd

    nc = build_program(meta)
    nc.compile()
    dedupe_ldweights(nc)
    res = run_bass_kernel_spmd(nc, in_maps, core_ids=list(range(len(in_maps))),
                               trace=trace)
    return res


def unshard(meta, results):
    B = meta["mapping"].size
    out = np.zeros((B, H, W, C), np.float32)
    for cidx, r in enumerate(results):
        o = np.asarray(r["out"], np.float32).reshape(-1)
        for j in range(meta["slots"]):
            b = meta["mapping"][cidx, j]
            WO = meta["wout"][j]
            t = o[meta["out_base"][j]:meta["out_base"][j] + 2 * 112 * WO]
            t = t.reshape(2, 112, WO)
            img = np.zeros((H, WC), np.float32)
            om = meta["omega"][cidx, j]
            for hb in (0, 1):
                for r_ in range(112):
                    u = 3 * int(om[hb, r_])
                    img[112 * hb + r_] = t[hb, r_, u:u + WC]
            img = img.reshape(H, W, C)
            if meta["transposed"][b]:
                img = img.transpose(1, 0, 2)
            out[b] = img
    return out


def kernel(x, kernels_table, amt, angles):
    x = np.asarray(x, np.float32)
    kernels_table = np.asarray(kernels_table, np.float32)
    amt = np.asarray(amt)
    angles = np.asarray(angles)
    meta, in_maps = prepare_host(x, kernels_table, amt, angles)
    res = run_cores(meta, in_maps)
    return unshard(meta, res.results)
